# revision 11
# baseline (speedup 1.0000x reference)
"""Depthwise causal Conv1d (k=4) + SiLU on 8 Trainium2 NeuronCores.

Problem: x [4, 4096, 2048] f32, w [2048, 4] f32,
out[b, t, d] = silu(sum_j w[d, j] * x[b, t - 3 + j, d])   (zero-padded left).

Sharding: 8 cores = 4 batches x 2 channel-halves. Depthwise conv is
independent per channel, so channel sharding needs no halo exchange.

Layout: each core receives its shard host-transposed to [channels, time]
(channels on SBUF partitions). The per-channel weight w[d, j] is then a
per-partition scalar and the causal time shifts are free-dim AP offsets
into one loaded tile.

The kernel is HBM-bound: 8.4 MB in + 8.4 MB out per core in fp16 is
~47 us at the ~22 GB/s-per-queue x 16-queue DMA roof, so the span is
set by how tightly the queues pack. Structure that matters:
 - The whole x shard (66 KB/partition) is prefetched. Load DEPENDENCIES
   are tracked per HWDGE ring as cumulative completion counts, so loads
   are emitted in consumption order and each consumer's wait threshold
   covers only loads up to its own block.
 - HWDGE rings hold ~4 outstanding triggers; each DIRECT2D trigger
   costs ~0.7 us of sequencer time. Early loads go on the SyncE ring in
   consumption order; late blocks (x3/x5/x7) are triggered from the ACT
   sequencer *between* early SiLUs, so their descriptors queue behind
   the early blocks and never steal ramp bandwidth (SWDGE loads would
   round-robin against HWDGE and delay x0 -- keep GpSimd stores-only).
 - Work split by measured rates (PE 7.3 us/block, DVE 12 us/block):
   blocks {0,2,3,5,6} + chunk 0 of 7 on the TensorEngine as diag(w_j)
   matmuls accumulating 4 taps in PSUM (diags built on-chip from a
   32 KB identity x per-partition weight, not a 1 MB host diag);
   blocks {1,4} + the rest of 7 elementwise on DVE: 4 tensor_scalar
   products (2x fp16 mode, ~400 G elem/s) + pair-packed adds (1x,
   ~235 G elem/s). scalar_tensor_tensor would fuse mul+add but runs
   ~4x slower than tensor_scalar -- do not use it.
 - ACT does only the SiLUs; stores issue on GpSimd (SWDGE) so a store
   blocked on its SiLU never head-of-line-blocks load issue.

Precision: x and the output are host-cast fp16 (halves HBM traffic both
ways and enables the fast DVE tensor_scalar mode); the DVE add tree
stays fp16, PE accumulates fp32 in PSUM; SiLU computes fp32-internally
on ACT. End-to-end relative error ~5e-4.
"""

import sys
import types

import numpy as np

import concourse.bass as bass
import concourse.bacc as bacc
import concourse.mybir as mybir
from concourse.tile import TileContext
from concourse.bass_utils import run_bass_kernel_spmd


def _ensure_ntff_hook():
    """bass_utils imports antenv.axon_hooks when BASS_TRACE is set; that
    module is absent on this image. Install a shim so tracing works when
    possible and degrades gracefully (instead of crashing) when not."""
    try:
        import antenv.axon_hooks  # noqa: F401

        return
    except ImportError:
        pass
    try:
        import antenv

        hook = None
        try:
            if "/root/.axon_site" not in sys.path:
                sys.path.insert(0, "/root/.axon_site")
            from trn_agent_boot.trn_boot import _ntff_profile_via_ctypes

            hook = _ntff_profile_via_ctypes("/opt/axon/libaxon_pjrt.so")
        except Exception:
            hook = None
        mod = types.ModuleType("antenv.axon_hooks")
        mod._hook = hook
        mod.get_axon_ntff_profile_hook = lambda: mod._hook
        mod.set_axon_ntff_profile_hook = lambda h: setattr(mod, "_hook", h)
        sys.modules["antenv.axon_hooks"] = mod
        antenv.axon_hooks = mod
    except Exception:
        pass


_ensure_ntff_hook()

B, L, D = 4, 4096, 2048
K = 4
PAD = K - 1
N_CORES = 8
DH = D // 2            # channels per core
NBLK = DH // 128       # 128-partition channel blocks per core
ROWW = 4128            # DRAM row stride (fp16 elems): 64B-aligned rows

MID_DT = mybir.dt.float16
PE_BLKS = (0, 2, 3, 5, 6)   # full blocks on the TensorEngine
EL_BLKS = (1, 4)            # full blocks elementwise on DVE
SPLIT_BLK = 7               # [0,1024) on PE, [1024,4096) on DVE
CW = 2048                   # elementwise chunk width
PQ = 2048                   # PE PSUM chunk width (4 banks, 2 bufs)

_cache = {}


def _build_bass():
    nc = bacc.Bacc()
    xt = nc.dram_tensor("xt", [DH, ROWW], MID_DT, kind="ExternalInput")
    wt = nc.dram_tensor("wt", [128, NBLK * K], mybir.dt.float32, kind="ExternalInput")
    ident = nc.dram_tensor("ident", [128, 128], MID_DT, kind="ExternalInput")
    ot = nc.dram_tensor("ot", [DH, L], MID_DT, kind="ExternalOutput")
    f32 = mybir.dt.float32

    with TileContext(nc) as tc:
        with tc.tile_pool(name="pool", bufs=2) as pool, \
             tc.tile_pool(name="psum", bufs=2, space="PSUM") as psum_pool:
            # wt + ident first on ACT's ring (the diag builds wait on the
            # scalar-ring count through these two), then a tiny warmup
            # Silu to force the silu activation-table load right away.
            w = pool.tile([128, NBLK * K], f32, tag="w", bufs=1)
            nc.scalar.dma_start(out=w[:], in_=wt[:, :])
            idt = pool.tile([128, 128], MID_DT, tag="id", bufs=1)
            nc.scalar.dma_start(out=idt[:], in_=ident[:, :])
            warm = pool.tile([128, 2], MID_DT, tag="warm", bufs=1)
            nc.vector.memset(warm[:], 0.0)
            nc.scalar.activation(warm[:], warm[:], mybir.ActivationFunctionType.Silu)

            # x tiles; early blocks loaded now on the SyncE ring in
            # consumption order (block 0 split so PE starts sooner).
            # x3/x5/x7 are triggered later from inside the ACT stream.
            x_tiles = {
                blk: pool.tile(
                    [128, L + PAD + 1], MID_DT, tag=f"x{blk}", bufs=1,
                    name=f"x{blk}",
                )
                for blk in range(NBLK)
            }

            def load(blk, eng, lo=0, hi=L + PAD):
                r0 = blk * 128
                eng.dma_start(out=x_tiles[blk][:, lo:hi], in_=xt[r0 : r0 + 128, lo:hi])

            # All x loads on the SyncE ring in consumption order. The ring
            # holds ~4 outstanding triggers, so triggers 5+ self-pace
            # against completions, keeping descriptor order = consumption
            # order (the scheduler preserves same-engine DMA order but
            # would hoist triggers placed on busy sequencers).
            load(0, nc.sync, 0, L // 2 + PAD)
            load(0, nc.sync, L // 2 + PAD, L + PAD)
            for blk in range(1, NBLK):
                load(blk, nc.sync)

            # diag(w_j) stationary operands for the PE path, built on-chip:
            # diag[p, m] = ident[p, m] * w[p, j]. Block 0's first so the
            # TensorEngine can start as soon as its x half lands.
            diags = {}
            for blk in PE_BLKS + (SPLIT_BLK,):
                dg = pool.tile([128, K * 128], MID_DT, tag=f"dg{blk}", bufs=1)
                diags[blk] = dg
                for j in range(K):
                    nc.vector.tensor_scalar_mul(
                        dg[:, j * 128 : (j + 1) * 128],
                        idt[:],
                        w[:, blk * K + j : blk * K + j + 1],
                    )

            def pe_chunk(blk, h0, pw=PQ):
                r0 = blk * 128
                x, dg = x_tiles[blk], diags[blk]
                ps = psum_pool.tile([128, PQ], f32, tag="ps", bufs=2)
                for j in range(K):
                    lw = dg[:, j * 128 : (j + 1) * 128]
                    for c in range(pw // 512):
                        nc.tensor.matmul(
                            ps[:, c * 512 : (c + 1) * 512],
                            lw,
                            x[:, h0 + c * 512 + j : h0 + c * 512 + j + 512],
                            start=(j == 0),
                            stop=(j == K - 1),
                        )
                o = pool.tile([128, PQ], MID_DT, tag="ope", bufs=3)
                nc.scalar.activation(
                    o[:, 0:pw], ps[:, 0:pw], mybir.ActivationFunctionType.Silu
                )
                nc.gpsimd.dma_start(
                    out=ot[r0 : r0 + 128, h0 : h0 + pw], in_=o[:, 0:pw]
                )

            def el_chunk(blk, t0, tl):
                r0 = blk * 128
                x = x_tiles[blk]
                wj = lambda j: w[:, blk * K + j : blk * K + j + 1]
                # qe holds the even-shift products [q0 | q2], qo the odd
                # [q1 | q3], each one contiguous [128, 2, tl] tile so both
                # pair-adds run as a single tensor_tensor op. Products are
                # shift-rebased: q_j[:, t] = w_j * x[:, t + j].
                qe = pool.tile([128, 2, CW], MID_DT, tag="qe", bufs=2)
                qo = pool.tile([128, 2, CW], MID_DT, tag="qo", bufs=2)
                nc.vector.tensor_scalar_mul(qe[:, 0, 0:tl], x[:, t0 : t0 + tl], wj(0))
                nc.vector.tensor_scalar_mul(
                    qo[:, 0, 0:tl], x[:, t0 + 1 : t0 + 1 + tl], wj(1)
                )
                nc.vector.tensor_scalar_mul(
                    qe[:, 1, 0:tl], x[:, t0 + 2 : t0 + 2 + tl], wj(2)
                )
                nc.vector.tensor_scalar_mul(
                    qo[:, 1, 0:tl], x[:, t0 + 3 : t0 + 3 + tl], wj(3)
                )
                if tl == CW:
                    nc.vector.tensor_add(qe[:, :, :], qe[:, :, :], qo[:, :, :])
                else:
                    nc.vector.tensor_add(
                        qe[:, 0, 0:tl], qe[:, 0, 0:tl], qo[:, 0, 0:tl]
                    )
                    nc.vector.tensor_add(
                        qe[:, 1, 0:tl], qe[:, 1, 0:tl], qo[:, 1, 0:tl]
                    )
                nc.vector.tensor_add(qe[:, 0, 0:tl], qe[:, 0, 0:tl], qe[:, 1, 0:tl])
                o = pool.tile([128, CW], MID_DT, tag="oel", bufs=4)
                nc.scalar.activation(
                    o[:, 0:tl], qe[:, 0, 0:tl], mybir.ActivationFunctionType.Silu
                )
                nc.gpsimd.dma_start(out=ot[r0 : r0 + 128, t0 : t0 + tl], in_=o[:, 0:tl])

            for blk in [0, 1, 2, 3, 4, 5, 6, 7]:
                if blk in PE_BLKS:
                    for chunk in range(L // PQ):
                        pe_chunk(blk, chunk * PQ)
                elif blk in EL_BLKS:
                    el_chunk(blk, 0, CW)
                    el_chunk(blk, CW, CW)
                else:  # SPLIT_BLK: [0,1024) on PE, the rest elementwise
                    pe_chunk(blk, 0, pw=1024)
                    el_chunk(blk, 1024, CW)
                    el_chunk(blk, 1024 + CW, L - 1024 - CW)
    nc.compile()
    return nc


def _shard_inputs(x, w):
    ident = np.eye(128, dtype=np.float16)
    in_maps = []
    for core in range(N_CORES):
        b, half = divmod(core, 2)
        d0 = half * DH
        xt = np.zeros((DH, ROWW), dtype=np.float16)
        xt[:, PAD : PAD + L] = x[b, :, d0 : d0 + DH].T.astype(np.float16)
        # w rows for this shard, rearranged so partition p holds the K
        # weights of channel blk*128 + p at free cols [blk*K, blk*K + K)
        w_sh = w[d0 : d0 + DH].reshape(NBLK, 128, K)
        wt = (
            w_sh.transpose(1, 0, 2).reshape(128, NBLK * K).astype(np.float32)
        )
        in_maps.append(
            {
                "xt": np.ascontiguousarray(xt),
                "wt": np.ascontiguousarray(wt),
                "ident": ident,
            }
        )
    return in_maps


def kernel(x, w):
    x = np.asarray(x, dtype=np.float32)
    w = np.asarray(w, dtype=np.float32)
    assert x.shape == (B, L, D) and w.shape == (D, K)

    if "nc" not in _cache:
        _cache["nc"] = _build_bass()
    nc = _cache["nc"]

    in_maps = _shard_inputs(x, w)
    res = None
    for attempt in range(3):
        try:
            res = run_bass_kernel_spmd(nc, in_maps, core_ids=list(range(N_CORES)))
            break
        except Exception:
            if attempt == 2:
                raise
    _cache["last_results"] = res

    out = np.empty((B, L, D), dtype=np.float32)
    for core in range(N_CORES):
        b, half = divmod(core, 2)
        d0 = half * DH
        out[b, :, d0 : d0 + DH] = res.results[core]["ot"].T.astype(np.float32)
    return out
